# revision 4
# baseline (speedup 1.0000x reference)
"""GAT (4-layer, softmax over dim=1) Trainium2 Bass kernel.

Sharding: data-parallel over batch B=8 -> one batch element per NeuronCore,
zero collectives.

Per-core algorithm (N=2048 nodes, F=256 features, 4 layers):
  All large tensors are kept in "T layout": [j on partitions, i on free dim]
  so that the softmax (normalization over i for each column j in the reference,
  axis=1) becomes a free-axis reduction, and the output matmul
  out[i,o] = sum_j att[i,j] Wh[j,o] takes attT strips directly as lhsT.

  Per layer:
    hT [f, n] (SBUF resident)  ->  Wh = hT.T @ W  (PE)
    f1[n] = h @ (W a1), f2[n] = h @ (W a2)        (PE, tiny)
    f1 broadcast across partitions via ones-outer-product (PE)
    per j-strip [128 x 2048]:
      pm   = f1b + maskT_strip          (DVE tensor_tensor, bf16 mask {0,-500})
      t1   = Prelu(pm + f2[j], a=0.2)   (ACT; leakyrelu; -500 kills masked)
      expe = Exp(t1), accum_out -> s[j] (ACT; shift-free softmax, e is small)
      r = 1/s (DVE), Whs[j,:] = Wh[j,:] * r[j] (GpSimd)
      8 matmuls accumulate outT[o, i] += Whs_strip.T @ expe_strip (PE)
    tail: hT_next = Prelu(outT_psum) directly in [f, n] layout (ACT)
  Final layer accumulates in natural [i, o] layout and DMAs out.

Host prep: transposed x, transposed additive bf16 mask (adj is reused by all
4 layers; {0,-500} additive mask is exact through prelu+exp), W @ a1/2.
"""

import numpy as np
import ml_dtypes

import bass_rust
import concourse.bass as bass
import concourse.mybir as mybir
import concourse.tile as tile
from concourse.bass_utils import run_bass_kernel_spmd

f32 = mybir.dt.float32
bf16 = mybir.dt.bfloat16
AFT = mybir.ActivationFunctionType

B, N, F, L = 8, 2048, 256, 4
NT = N // 128  # 16 node tiles
FC = F // 128  # 2 feature chunks
IC = N // 512  # 4 i-chunks per strip
ALPHA = 0.2
MASKADD = -500.0


def split_multi_waits(nc):
    """This container's walrus supports at most one sync-wait per instruction;
    Tile's exit drain (and occasionally the scheduler) attaches several. Hoist
    extras onto same-engine EventSemaphore instructions placed just before."""
    for fn in nc.m.functions:
        for blk in fn.blocks:
            new_list, changed = [], False
            for inst in blk.instructions:
                si = inst.sync_info
                if si is not None and len(si.on_wait) > 1:
                    waits = list(si.on_wait)
                    for k, w in enumerate(waits[:-1]):
                        es = mybir.InstEventSemaphore(name=f"{inst.name}_wsplit{k}")
                        es.engine = inst.engine
                        es.sync_info = bass_rust.SyncInfo(on_wait=[w], on_update=[])
                        new_list.append(es)
                    si.on_wait = [waits[-1]]
                    changed = True
                new_list.append(inst)
            if changed:
                blk.instructions = new_list


def build_nc(do_split=True):
    nc = bass.Bass()
    xT_d = nc.dram_tensor("xT", [F, N], f32, kind="ExternalInput")
    mask_d = nc.dram_tensor("maskT", [N, N], bf16, kind="ExternalInput")
    W_d = nc.dram_tensor("W", [L, F, F], f32, kind="ExternalInput")
    wa_d = nc.dram_tensor("wa", [L, F, 2], f32, kind="ExternalInput")
    ones_d = nc.dram_tensor("ones", [1, 128], f32, kind="ExternalInput")
    ident_d = nc.dram_tensor("ident", [128, 128], f32, kind="ExternalInput")
    out_d = nc.dram_tensor("out", [N, F], f32, kind="ExternalOutput")

    with tile.TileContext(nc) as tc:
        with (
            tc.tile_pool(name="const", bufs=1) as constp,
            tc.tile_pool(name="hT", bufs=2) as hTp,
            tc.tile_pool(name="wl", bufs=2) as wlp,
            tc.tile_pool(name="wh", bufs=1) as whp,
            tc.tile_pool(name="fvec", bufs=2) as fvp,
            tc.tile_pool(name="strip", bufs=2) as stripp,
            tc.tile_pool(name="sr", bufs=4) as srp,
            tc.tile_pool(name="whs", bufs=3) as whsp,
            tc.tile_pool(name="outsb", bufs=3) as outp,
            tc.tile_pool(name="bank", bufs=8, space="PSUM") as psp,
        ):
            ones_sb = constp.tile([1, 128], f32)
            nc.sync.dma_start(ones_sb[:], ones_d[:])
            ident_sb = constp.tile([128, 128], f32)
            nc.sync.dma_start(ident_sb[:], ident_d[:])
            mask_sb = constp.tile([128, NT * N], bf16)
            for jt in range(NT):
                nc.sync.dma_start(
                    mask_sb[:, jt * N : (jt + 1) * N],
                    mask_d[jt * 128 : (jt + 1) * 128, :],
                )
            hT_cur = hTp.tile([128, FC * N], f32, tag="hT")
            for fc in range(FC):
                nc.sync.dma_start(
                    hT_cur[:, fc * N : (fc + 1) * N],
                    xT_d[fc * 128 : (fc + 1) * 128, :],
                )

            for l in range(L):
                W_sb = wlp.tile([128, FC * F], f32, tag="W")
                wa_sb = wlp.tile([128, FC * 2], f32, tag="wa")
                for fc in range(FC):
                    nc.sync.dma_start(
                        W_sb[:, fc * F : (fc + 1) * F],
                        W_d[l, fc * 128 : (fc + 1) * 128, :],
                    )
                    nc.sync.dma_start(
                        wa_sb[:, fc * 2 : (fc + 1) * 2],
                        wa_d[l, fc * 128 : (fc + 1) * 128, :],
                    )

                # ---- f-phase ----
                f12_sb = fvp.tile([128, NT * 2], f32, tag="f12")
                for nt in range(NT):
                    ps = psp.tile([128, 512], f32, tag="bank")
                    for fc in range(FC):
                        nc.tensor.matmul(
                            ps[:, 0:2],
                            hT_cur[:, fc * N + nt * 128 : fc * N + (nt + 1) * 128],
                            wa_sb[:, fc * 2 : (fc + 1) * 2],
                            start=(fc == 0),
                            stop=(fc == FC - 1),
                        )
                    nc.vector.tensor_copy(f12_sb[:, nt * 2 : nt * 2 + 2], ps[:, 0:2])

                f1row_sb = fvp.tile([1, N], f32, tag="f1row")
                for ic in range(IC):
                    ps = psp.tile([128, 512], f32, tag="bank")
                    for fc in range(FC):
                        nc.tensor.matmul(
                            ps[0:1, :],
                            wa_sb[:, fc * 2 : fc * 2 + 1],
                            hT_cur[:, fc * N + ic * 512 : fc * N + (ic + 1) * 512],
                            start=(fc == 0),
                            stop=(fc == FC - 1),
                        )
                    nc.scalar.copy(
                        f1row_sb[0:1, ic * 512 : (ic + 1) * 512], ps[0:1, :]
                    )

                f1b = fvp.tile([128, N], f32, tag="f1b")
                for ic in range(IC):
                    ps = psp.tile([128, 512], f32, tag="bank")
                    nc.tensor.matmul(
                        ps[:, :],
                        ones_sb[0:1, :],
                        f1row_sb[0:1, ic * 512 : (ic + 1) * 512],
                        start=True,
                        stop=True,
                    )
                    nc.vector.tensor_copy(f1b[:, ic * 512 : (ic + 1) * 512], ps[:, :])

                Wh_sb = whp.tile([128, NT * F], f32, tag="Wh")
                for nt in range(NT):
                    ps = psp.tile([128, 512], f32, tag="bank")
                    for fc in range(FC):
                        nc.tensor.matmul(
                            ps[:, 0:F],
                            hT_cur[:, fc * N + nt * 128 : fc * N + (nt + 1) * 128],
                            W_sb[:, fc * F : (fc + 1) * F],
                            start=(fc == 0),
                            stop=(fc == FC - 1),
                        )
                    if nt % 2 == 0:
                        nc.scalar.copy(Wh_sb[:, nt * F : (nt + 1) * F], ps[:, 0:F])
                    else:
                        nc.vector.tensor_copy(
                            Wh_sb[:, nt * F : (nt + 1) * F], ps[:, 0:F]
                        )

                # ---- strip loop ----
                psum_out = [
                    psp.tile([128, 512], f32, tag="bank", name=f"po_{l}_{k}")
                    for k in range(8)
                ]
                for jt in range(NT):
                    pm = stripp.tile([128, N], f32, tag="pm")
                    nc.vector.tensor_tensor(
                        pm[:, :],
                        f1b[:, :],
                        mask_sb[:, jt * N : (jt + 1) * N],
                        mybir.AluOpType.add,
                    )
                    t1 = stripp.tile([128, N], f32, tag="t1")
                    nc.scalar.activation(
                        t1[:, :],
                        pm[:, :],
                        AFT.Prelu,
                        bias=f12_sb[:, jt * 2 + 1 : jt * 2 + 2],
                        scale=1.0,
                        alpha=ALPHA,
                    )
                    expe = stripp.tile([128, N], f32, tag="expe")
                    s_t = srp.tile([128, 1], f32, tag="s")
                    nc.scalar.activation(
                        expe[:, :], t1[:, :], AFT.Exp, accum_out=s_t[:, :]
                    )
                    r_t = srp.tile([128, 1], f32, tag="r")
                    nc.vector.reciprocal(r_t[:, :], s_t[:, :])
                    whs_t = whsp.tile([128, F], f32, tag="whs")
                    nc.gpsimd.tensor_scalar_mul(
                        whs_t[:, :], Wh_sb[:, jt * F : (jt + 1) * F], r_t[:, :]
                    )
                    for oc in range(2):
                        for ic in range(IC):
                            nc.tensor.matmul(
                                psum_out[oc * IC + ic][:, :],
                                whs_t[:, oc * 128 : (oc + 1) * 128],
                                expe[:, ic * 512 : (ic + 1) * 512],
                                start=(jt == 0),
                                stop=(jt == NT - 1),
                            )

                # ---- tail ----
                hT_next = hTp.tile([128, FC * N], f32, tag="hT")
                for oc in range(2):
                    for ic in range(IC):
                        nc.scalar.activation(
                            hT_next[:, oc * N + ic * 512 : oc * N + (ic + 1) * 512],
                            psum_out[oc * IC + ic][:, :],
                            AFT.Prelu,
                            alpha=ALPHA,
                        )
                if l < L - 1:
                    hT_cur = hT_next
                else:
                    # transpose houtT [o, i] -> out [i, o] via PE identity matmuls
                    for nt in range(NT):
                        ob = outp.tile([128, F], f32, tag="ob")
                        for oc in range(FC):
                            pst = psp.tile([128, 512], f32, tag="bank", name=f"tr_{nt}_{oc}")
                            nc.tensor.matmul(
                                pst[:, 0:128],
                                hT_next[:, oc * N + nt * 128 : oc * N + (nt + 1) * 128],
                                ident_sb[:, :],
                                start=True,
                                stop=True,
                            )
                            if oc % 2 == 0:
                                nc.scalar.copy(
                                    ob[:, oc * 128 : (oc + 1) * 128], pst[:, 0:128]
                                )
                            else:
                                nc.vector.tensor_copy(
                                    ob[:, oc * 128 : (oc + 1) * 128], pst[:, 0:128]
                                )
                        nc.sync.dma_start(out_d[nt * 128 : (nt + 1) * 128, :], ob[:, :])

    if do_split:
        split_multi_waits(nc)
    return nc


_NC = None


def _get_nc():
    global _NC
    if _NC is None:
        _NC = build_nc()
    return _NC


def _host_prep(x, adj, W0, Wrest, A):
    x = np.asarray(x, dtype=np.float32)
    adj = np.asarray(adj)
    W_all = np.stack(
        [np.asarray(W0, dtype=np.float32)]
        + [np.asarray(Wrest[i], dtype=np.float32) for i in range(L - 1)]
    )  # [4, F, F]
    A = np.asarray(A, dtype=np.float32)
    wa = np.empty((L, F, 2), dtype=np.float32)
    for l in range(L):
        wa[l, :, 0] = W_all[l] @ A[l, :F]
        wa[l, :, 1] = W_all[l] @ A[l, F:]
    ones = np.ones((1, 128), dtype=np.float32)
    ident = np.eye(128, dtype=np.float32)

    in_maps = []
    for b in range(B):
        xT = np.ascontiguousarray(x[b].T)
        adjT = adj[b].T.astype(np.float32)
        maskT = ((adjT - 1.0) * (-MASKADD)).astype(ml_dtypes.bfloat16)
        in_maps.append(
            {
                "xT": xT,
                "maskT": maskT,
                "W": W_all,
                "wa": wa,
                "ones": ones,
                "ident": ident,
            }
        )
    return in_maps


def kernel(x, adj, W0, Wrest, A, _trace=False, _trace_kwargs=None):
    nc = _get_nc()
    in_maps = _host_prep(x, adj, W0, Wrest, A)
    res = run_bass_kernel_spmd(
        nc,
        in_maps,
        core_ids=list(range(B)),
        trace=_trace,
        **(_trace_kwargs or {}),
    )
    out = np.stack([res.results[b]["out"] for b in range(B)])
    if _trace:
        kernel.last_exec_time_ns = res.exec_time_ns
        kernel.last_results = res
    return out


# revision 5
# speedup vs baseline: 1.0067x; 1.0067x over previous
"""GAT (4-layer, softmax over dim=1) Trainium2 Bass kernel.

Sharding: data-parallel over batch B=8 -> one batch element per NeuronCore,
zero collectives.

Per-core algorithm (N=2048 nodes, F=256 features, 4 layers):
  All large tensors are kept in "T layout": [j on partitions, i on free dim]
  so that the softmax (normalization over i for each column j in the reference,
  axis=1) becomes a free-axis reduction, and the output matmul
  out[i,o] = sum_j att[i,j] Wh[j,o] takes attT strips directly as lhsT.

  Per layer:
    hT [f, n] (SBUF resident)  ->  Wh = hT.T @ W  (PE)
    f1[n] = h @ (W a1), f2[n] = h @ (W a2)        (PE, tiny)
    f1 broadcast across partitions via ones-outer-product (PE)
    per j-strip [128 x 2048]:
      pm   = f1b + maskT_strip          (DVE tensor_tensor, bf16 mask {0,-500})
      t1   = Prelu(pm + f2[j], a=0.2)   (ACT; leakyrelu; -500 kills masked)
      expe = Exp(t1), accum_out -> s[j] (ACT; shift-free softmax, e is small)
      r = 1/s (DVE), Whs[j,:] = Wh[j,:] * r[j] (GpSimd)
      8 matmuls accumulate outT[o, i] += Whs_strip.T @ expe_strip (PE)
    tail: hT_next = Prelu(outT_psum) directly in [f, n] layout (ACT)
  Final layer accumulates in natural [i, o] layout and DMAs out.

Host prep: transposed x, transposed additive bf16 mask (adj is reused by all
4 layers; {0,-500} additive mask is exact through prelu+exp), W @ a1/2.
"""

import numpy as np
import ml_dtypes

import bass_rust
import concourse.bass as bass
import concourse.mybir as mybir
import concourse.tile as tile
from concourse.bass_utils import run_bass_kernel_spmd

f32 = mybir.dt.float32
bf16 = mybir.dt.bfloat16
AFT = mybir.ActivationFunctionType

B, N, F, L = 8, 2048, 256, 4
NT = N // 128  # 16 node tiles
FC = F // 128  # 2 feature chunks
IC = N // 512  # 4 i-chunks per strip
ALPHA = 0.2
MASKADD = -500.0


def split_multi_waits(nc):
    """This container's walrus supports at most one sync-wait per instruction;
    Tile's exit drain (and occasionally the scheduler) attaches several. Hoist
    extras onto same-engine EventSemaphore instructions placed just before."""
    for fn in nc.m.functions:
        for blk in fn.blocks:
            new_list, changed = [], False
            for inst in blk.instructions:
                si = inst.sync_info
                if si is not None and len(si.on_wait) > 1:
                    waits = list(si.on_wait)
                    for k, w in enumerate(waits[:-1]):
                        es = mybir.InstEventSemaphore(name=f"{inst.name}_wsplit{k}")
                        es.engine = inst.engine
                        es.sync_info = bass_rust.SyncInfo(on_wait=[w], on_update=[])
                        new_list.append(es)
                    si.on_wait = [waits[-1]]
                    changed = True
                new_list.append(inst)
            if changed:
                blk.instructions = new_list


def build_nc(do_split=True):
    nc = bass.Bass()
    xT_d = nc.dram_tensor("xT", [F, N], f32, kind="ExternalInput")
    mask_d = nc.dram_tensor("maskT", [N, N], bf16, kind="ExternalInput")
    W_d = nc.dram_tensor("W", [L, F, F], f32, kind="ExternalInput")
    wa_d = nc.dram_tensor("wa", [L, F, 2], f32, kind="ExternalInput")
    ones_d = nc.dram_tensor("ones", [1, 128], f32, kind="ExternalInput")
    ident_d = nc.dram_tensor("ident", [128, 128], f32, kind="ExternalInput")
    out_d = nc.dram_tensor("out", [N, F], f32, kind="ExternalOutput")

    with tile.TileContext(nc) as tc:
        with (
            tc.tile_pool(name="const", bufs=1) as constp,
            tc.tile_pool(name="hT", bufs=2) as hTp,
            tc.tile_pool(name="wl", bufs=2) as wlp,
            tc.tile_pool(name="wh", bufs=1) as whp,
            tc.tile_pool(name="fvec", bufs=2) as fvp,
            tc.tile_pool(name="strip", bufs=3) as stripp,
            tc.tile_pool(name="sr", bufs=4) as srp,
            tc.tile_pool(name="whs", bufs=3) as whsp,
            tc.tile_pool(name="outsb", bufs=3) as outp,
            tc.tile_pool(name="bank", bufs=8, space="PSUM") as psp,
        ):
            ones_sb = constp.tile([1, 128], f32)
            nc.sync.dma_start(ones_sb[:], ones_d[:])
            ident_sb = constp.tile([128, 128], f32)
            nc.sync.dma_start(ident_sb[:], ident_d[:])
            mask_sb = constp.tile([128, NT * N], bf16)
            for jt in range(NT):
                nc.sync.dma_start(
                    mask_sb[:, jt * N : (jt + 1) * N],
                    mask_d[jt * 128 : (jt + 1) * 128, :],
                )
            hT_cur = hTp.tile([128, FC * N], f32, tag="hT")
            for fc in range(FC):
                nc.sync.dma_start(
                    hT_cur[:, fc * N : (fc + 1) * N],
                    xT_d[fc * 128 : (fc + 1) * 128, :],
                )

            for l in range(L):
                W_sb = wlp.tile([128, FC * F], f32, tag="W")
                wa_sb = wlp.tile([128, FC * 2], f32, tag="wa")
                for fc in range(FC):
                    nc.sync.dma_start(
                        W_sb[:, fc * F : (fc + 1) * F],
                        W_d[l, fc * 128 : (fc + 1) * 128, :],
                    )
                    nc.sync.dma_start(
                        wa_sb[:, fc * 2 : (fc + 1) * 2],
                        wa_d[l, fc * 128 : (fc + 1) * 128, :],
                    )

                # ---- f-phase ----
                f12_sb = fvp.tile([128, NT * 2], f32, tag="f12")
                for nt in range(NT):
                    ps = psp.tile([128, 512], f32, tag="bank")
                    for fc in range(FC):
                        nc.tensor.matmul(
                            ps[:, 0:2],
                            hT_cur[:, fc * N + nt * 128 : fc * N + (nt + 1) * 128],
                            wa_sb[:, fc * 2 : (fc + 1) * 2],
                            start=(fc == 0),
                            stop=(fc == FC - 1),
                        )
                    nc.vector.tensor_copy(f12_sb[:, nt * 2 : nt * 2 + 2], ps[:, 0:2])

                f1row_sb = fvp.tile([1, N], f32, tag="f1row")
                for ic in range(IC):
                    ps = psp.tile([128, 512], f32, tag="bank")
                    for fc in range(FC):
                        nc.tensor.matmul(
                            ps[0:1, :],
                            wa_sb[:, fc * 2 : fc * 2 + 1],
                            hT_cur[:, fc * N + ic * 512 : fc * N + (ic + 1) * 512],
                            start=(fc == 0),
                            stop=(fc == FC - 1),
                        )
                    nc.scalar.copy(
                        f1row_sb[0:1, ic * 512 : (ic + 1) * 512], ps[0:1, :]
                    )

                f1b = fvp.tile([128, N], f32, tag="f1b")
                for ic in range(IC):
                    ps = psp.tile([128, 512], f32, tag="bank")
                    nc.tensor.matmul(
                        ps[:, :],
                        ones_sb[0:1, :],
                        f1row_sb[0:1, ic * 512 : (ic + 1) * 512],
                        start=True,
                        stop=True,
                    )
                    nc.vector.tensor_copy(f1b[:, ic * 512 : (ic + 1) * 512], ps[:, :])

                Wh_sb = whp.tile([128, NT * F], f32, tag="Wh")
                for nt in range(NT):
                    ps = psp.tile([128, 512], f32, tag="bank")
                    for fc in range(FC):
                        nc.tensor.matmul(
                            ps[:, 0:F],
                            hT_cur[:, fc * N + nt * 128 : fc * N + (nt + 1) * 128],
                            W_sb[:, fc * F : (fc + 1) * F],
                            start=(fc == 0),
                            stop=(fc == FC - 1),
                        )
                    if nt % 2 == 0:
                        nc.scalar.copy(Wh_sb[:, nt * F : (nt + 1) * F], ps[:, 0:F])
                    else:
                        nc.vector.tensor_copy(
                            Wh_sb[:, nt * F : (nt + 1) * F], ps[:, 0:F]
                        )

                # ---- strip loop ----
                psum_out = [
                    psp.tile([128, 512], f32, tag="bank", name=f"po_{l}_{k}")
                    for k in range(8)
                ]
                for jt in range(NT):
                    pm = stripp.tile([128, N], f32, tag="pm")
                    nc.vector.tensor_tensor(
                        pm[:, :],
                        f1b[:, :],
                        mask_sb[:, jt * N : (jt + 1) * N],
                        mybir.AluOpType.add,
                    )
                    nc.scalar.activation(
                        pm[:, :],
                        pm[:, :],
                        AFT.Prelu,
                        bias=f12_sb[:, jt * 2 + 1 : jt * 2 + 2],
                        scale=1.0,
                        alpha=ALPHA,
                    )
                    expe = stripp.tile([128, N], f32, tag="expe")
                    s_t = srp.tile([128, 1], f32, tag="s")
                    nc.scalar.activation(
                        expe[:, :], pm[:, :], AFT.Exp, accum_out=s_t[:, :]
                    )
                    r_t = srp.tile([128, 1], f32, tag="r")
                    nc.vector.reciprocal(r_t[:, :], s_t[:, :])
                    whs_t = whsp.tile([128, F], f32, tag="whs")
                    nc.vector.tensor_scalar_mul(
                        whs_t[:, :], Wh_sb[:, jt * F : (jt + 1) * F], r_t[:, :]
                    )
                    for oc in range(2):
                        for ic in range(IC):
                            nc.tensor.matmul(
                                psum_out[oc * IC + ic][:, :],
                                whs_t[:, oc * 128 : (oc + 1) * 128],
                                expe[:, ic * 512 : (ic + 1) * 512],
                                start=(jt == 0),
                                stop=(jt == NT - 1),
                            )

                # ---- tail ----
                hT_next = hTp.tile([128, FC * N], f32, tag="hT")
                for oc in range(2):
                    for ic in range(IC):
                        nc.scalar.activation(
                            hT_next[:, oc * N + ic * 512 : oc * N + (ic + 1) * 512],
                            psum_out[oc * IC + ic][:, :],
                            AFT.Prelu,
                            alpha=ALPHA,
                        )
                if l < L - 1:
                    hT_cur = hT_next
                else:
                    # transpose houtT [o, i] -> out [i, o] via PE identity matmuls
                    for nt in range(NT):
                        ob = outp.tile([128, F], f32, tag="ob")
                        for oc in range(FC):
                            pst = psp.tile([128, 512], f32, tag="bank", name=f"tr_{nt}_{oc}")
                            nc.tensor.matmul(
                                pst[:, 0:128],
                                hT_next[:, oc * N + nt * 128 : oc * N + (nt + 1) * 128],
                                ident_sb[:, :],
                                start=True,
                                stop=True,
                            )
                            if oc % 2 == 0:
                                nc.scalar.copy(
                                    ob[:, oc * 128 : (oc + 1) * 128], pst[:, 0:128]
                                )
                            else:
                                nc.vector.tensor_copy(
                                    ob[:, oc * 128 : (oc + 1) * 128], pst[:, 0:128]
                                )
                        nc.sync.dma_start(out_d[nt * 128 : (nt + 1) * 128, :], ob[:, :])

    if do_split:
        split_multi_waits(nc)
    return nc


_NC = None


def _get_nc():
    global _NC
    if _NC is None:
        _NC = build_nc()
    return _NC


def _host_prep(x, adj, W0, Wrest, A):
    x = np.asarray(x, dtype=np.float32)
    adj = np.asarray(adj)
    W_all = np.stack(
        [np.asarray(W0, dtype=np.float32)]
        + [np.asarray(Wrest[i], dtype=np.float32) for i in range(L - 1)]
    )  # [4, F, F]
    A = np.asarray(A, dtype=np.float32)
    wa = np.empty((L, F, 2), dtype=np.float32)
    for l in range(L):
        wa[l, :, 0] = W_all[l] @ A[l, :F]
        wa[l, :, 1] = W_all[l] @ A[l, F:]
    ones = np.ones((1, 128), dtype=np.float32)
    ident = np.eye(128, dtype=np.float32)

    in_maps = []
    for b in range(B):
        xT = np.ascontiguousarray(x[b].T)
        adjT = adj[b].T.astype(np.float32)
        maskT = ((adjT - 1.0) * (-MASKADD)).astype(ml_dtypes.bfloat16)
        in_maps.append(
            {
                "xT": xT,
                "maskT": maskT,
                "W": W_all,
                "wa": wa,
                "ones": ones,
                "ident": ident,
            }
        )
    return in_maps


def kernel(x, adj, W0, Wrest, A, _trace=False, _trace_kwargs=None):
    nc = _get_nc()
    in_maps = _host_prep(x, adj, W0, Wrest, A)
    res = run_bass_kernel_spmd(
        nc,
        in_maps,
        core_ids=list(range(B)),
        trace=_trace,
        **(_trace_kwargs or {}),
    )
    out = np.stack([res.results[b]["out"] for b in range(B)])
    if _trace:
        kernel.last_exec_time_ns = res.exec_time_ns
        kernel.last_results = res
    return out
